# revision 26
# baseline (speedup 1.0000x reference)
"""Bass/Trainium2 kernel for a 2-layer bidirectional LSTM (Keras semantics).

Problem: B=1024, T=200, D=U=128, 2 layers, merge_mode='ave', biases all 1.0.

Sharding: data-parallel over batch across 8 cores (Bc=128 per core).
Each core runs all 4 LSTM passes (fw/bw x 2 layers) on its batch slice as
two concurrent layer-pair wavefronts: the layer-2 pair lags the layer-1
pair by LAG steps, so both recurrences advance in parallel and fill each
other's pipeline gaps.

Layout: feature-major ("transposed") everywhere on device.  Hidden state h
is kept as [U, batch] tiles so it feeds the next step's matmul as the
stationary operand without any per-step transposes.  Each layer-pair's gate
pre-activations live in their own PSUM banks, double-buffered by step
parity, and the input projections are issued one step ahead (they do not
depend on the recurrent state) so only the 8 recurrent matmuls sit on each
step's serial chain.

The host pre-transposes x to [D, T, Bc] and pre-casts x/weights to bf16;
matmuls run bf16 with fp32 PSUM accumulation; cell state c stays fp32.
"""

import numpy as np

import concourse.bacc as bacc
import concourse.mybir as mybir
import concourse.tile as tile

B, T, D, U = 1024, 200, 128, 128
NCORES = 8
BC = B // NCORES
LAG = 4  # layer-2 wavefront lag in steps (must be >= 2)

F32 = mybir.dt.float32
BF16 = mybir.dt.bfloat16
SIGMOID = mybir.ActivationFunctionType.Sigmoid
TANH = mybir.ActivationFunctionType.Tanh
MULT = mybir.AluOpType.mult

# Units in the shared PSUM tile: 0=l2.fw 1=l2.bw 2=l1.fw 3=l1.bw.
# Gate bank order per unit is [i, f, o, g]; the host pre-permutes the
# source weights (gate order i,f,g,o) into this bank order.
GATE_SRC = [0, 1, 3, 2]

_CACHE = {}
DEBUG_L1 = False
REPEAT = 1  # emit the whole computation N times (device-time measurement)
SIG_SPLIT = True  # sigmoid [i,f] separate from [o]: shorter critical chain
# TANH_FORM: compute every gate with a single Tanh activation using
# sigmoid(y) = (tanh(y/2)+1)/2.  The per-gate argument scaling (y/2 for
# i,f,o) is folded into the weights on the host; the per-gate bias
# (0.5/0.5/0.5/1.0) is pre-added into PSUM by a K=1 matmul against a
# constant pattern; the (t+1)/2 fix-ups ride inside scalar_tensor_tensor
# ops, and h is stored doubled (h~ = 2h) with the compensation folded into
# the consuming weights and the host's final merge scale (0.25).
TANH_FORM = True
T_PER_UNIT = True  # one Tanh per unit (shorter chain) vs per pair (fewer insts)
L1_FIRST = False  # emit layer-1 pair before layer-2 within an iteration


def _emit(nc, tc, ctx, x_in, wk_in, wrk_in, out, out1=None):
    consts = ctx.enter_context(tc.tile_pool(name="consts", bufs=1))
    bigs = ctx.enter_context(tc.tile_pool(name="bigs", bufs=1))
    work = ctx.enter_context(tc.tile_pool(name="work", bufs=2))
    psum = ctx.enter_context(tc.tile_pool(name="psum", bufs=1, space="PSUM"))

    # Weights: [unit, D, 4U] with gates pre-permuted to [i,f,o,g] by the host.
    wk = consts.tile([128, 4, 4 * U], BF16, tag="wk")
    wrk = consts.tile([128, 4, 4 * U], BF16, tag="wrk")
    nc.sync.dma_start(wk[:], wk_in.rearrange("u p c -> p u c"))
    nc.sync.dma_start(wrk[:], wrk_in.rearrange("u p c -> p u c"))

    if TANH_FORM:
        # K=1 matmul operands for the per-gate-column bias pattern.
        ones1 = consts.tile([1, 128], BF16, tag="ones1")
        biasp = consts.tile([1, 4 * U], BF16, tag="biasp")
        nc.gpsimd.memset(ones1[:], 1.0)
        nc.gpsimd.memset(biasp[:, 0 : 3 * U], 0.5)
        nc.gpsimd.memset(biasp[:, 3 * U : 4 * U], 1.0)

    # Big persistent buffers.
    xT = bigs.tile([128, T, BC], BF16, tag="xT")          # x, feature-major
    hbuf = bigs.tile([128, T, 2, BC], BF16, tag="hbuf")   # l1 h, overwritten in place by l2 h
    gate_ps = psum.tile([128, 2, 4, 4 * U], F32, tag="ps")  # [parity, unit, i|f|o|g]

    # Input DMA: front/back interleaved 8-step chunks, emitted ahead of use.
    CH = 8
    chunks = []
    fr, bk = 0, T - CH
    while fr < bk:
        chunks.append(fr)
        chunks.append(bk)
        fr += CH
        bk -= CH
    if fr == bk:
        chunks.append(fr)

    def emit_x_chunk(ci):
        if ci < len(chunks):
            t0 = chunks[ci]
            nc.sync.dma_start(xT[:, t0 : t0 + CH, :], x_in[:, t0 : t0 + CH, :])

    XAHEAD = 4
    for ci in range(XAHEAD):
        emit_x_chunk(ci)

    # pair id 0 = layer 2 (units 0,1), pair id 1 = layer 1 (units 2,3)
    def proj_rhs(pid, u, t):
        if pid == 1:
            return xT[:, t, :] if u == 2 else xT[:, T - 1 - t, :]
        return hbuf[:, t, u, :]

    def emit_proj(pid, t, par):
        """Input projections for pair `pid` step `t` into parity `par`.

        The whole bank (4 gate projections + 4 recurrent matmuls of the same
        step) forms ONE psum accumulation group: start on the first proj,
        stop on the last rec (or last proj for step 0, which has no recs).
        """
        units = (2, 3) if pid == 1 else (0, 1)
        for u in units:
            rhs = proj_rhs(pid, u, t)
            for g in range(4):
                dst = gate_ps[:, par, u, g * U : (g + 1) * U]
                w = slice(g * U, (g + 1) * U)
                stop = t == 0 and g == 3 and not TANH_FORM
                nc.tensor.matmul(
                    dst, wk[:, u, w], rhs, start=(g == 0), stop=stop
                )
            if TANH_FORM:
                nc.tensor.matmul(
                    gate_ps[:, par, u, :],
                    ones1[:],
                    biasp[:],
                    start=False,
                    stop=(t == 0),
                )

    def emit_pair_step(pid, t, p):
        units = (2, 3) if pid == 1 else (0, 1)
        ulo = units[0]
        tag = f"p{pid}"

        # --- recurrent matmuls: the only PE work on the step's serial chain.
        if t > 0:
            for u in units:
                rhs = hbuf[:, t - 1, u - ulo, :]
                for g in range(4):
                    dst = gate_ps[:, p, u, g * U : (g + 1) * U]
                    w = slice(g * U, (g + 1) * U)
                    nc.tensor.matmul(
                        dst, wrk[:, u, w], rhs, start=False, stop=(g == 3)
                    )

        if TANH_FORM:
            # One Tanh covers all four gates: i',f',o' are tanh-form
            # sigmoids ((t+1)/2 folded into STT ops below), g is final.
            th = work.tile([128, 2, 4 * U], F32, tag="th" + tag, bufs=3)
            ps_pair = gate_ps[:, p, ulo : ulo + 2, :]
            ADD = mybir.AluOpType.add
            # s = 2*c = (f'+1)*c_prev + (i'+1)*g, computed per unit so each
            # unit's cell math starts right after its own Tanh.
            s_t = work.tile([128, 2, U], F32, tag="s" + tag, bufs=3)
            c_prev = _CACHE["c_prev" + tag] if t > 0 else None
            if t > 0:
                e1 = work.tile([128, 2, U], F32, tag="e1" + tag)
                e2 = work.tile([128, 2, U], F32, tag="e2" + tag)
            if not T_PER_UNIT:
                nc.scalar.activation(th[:], ps_pair[:], TANH)
            for ui in range(2):
                if T_PER_UNIT:
                    nc.scalar.activation(th[:, ui, :], ps_pair[:, ui, :], TANH)
                ip = th[:, ui, 0:U]
                fp_ = th[:, ui, U : 2 * U]
                gp = th[:, ui, 3 * U : 4 * U]
                su = s_t[:, ui, :]
                if t == 0:
                    nc.vector.scalar_tensor_tensor(su, ip, 1.0, gp, ADD, MULT)
                else:
                    # STT is DVE-only (walrus rejects it on Pool); the plain
                    # add runs on GPSIMD to keep DVE off the chain tail.
                    nc.vector.scalar_tensor_tensor(
                        e2[:, ui, :], ip, 1.0, gp, ADD, MULT
                    )
                    nc.vector.scalar_tensor_tensor(
                        e1[:, ui, :], fp_, 1.0, c_prev[:, ui, :], ADD, MULT
                    )
                    nc.gpsimd.tensor_add(su, e1[:, ui, :], e2[:, ui, :])
            _CACHE["th" + tag] = th
            _CACHE["s" + tag] = s_t
            return

    def emit_pair_tail(pid, t, p):
        """Phase B of a step: tanh(c), h, c.  Emitted after BOTH pairs'
        phase A so the other pair's Tanh instructions cover this pair's
        DVE/GPSIMD cell-chain latency on the in-order ACT stream."""
        tag = f"p{pid}"
        th = _CACHE["th" + tag]
        s_t = _CACHE["s" + tag]
        ADD = mybir.AluOpType.add
        c_new = work.tile([128, 2, U], F32, tag="c" + tag)
        tanc = work.tile([128, 2, U], F32, tag="tanc" + tag, bufs=3)
        nc.scalar.activation(tanc[:], s_t[:], TANH, scale=0.5)
        # h~ = 2h = (o'+1)*tanh(c) -> bf16 layer output / next input,
        # written per unit so each unit's next rec matmuls start as soon
        # as its own half is stored.
        for ui in range(2):
            nc.vector.scalar_tensor_tensor(
                hbuf[:, t, ui, :],
                th[:, ui, 2 * U : 3 * U],
                1.0,
                tanc[:, ui, :],
                ADD,
                MULT,
            )
        # true c for the next step (off the h critical path, on the
        # otherwise-idle GPSIMD engine)
        nc.gpsimd.tensor_scalar_mul(c_new[:], s_t[:], 0.5)
        _CACHE["c_prev" + tag] = c_new
        # --- prefetch next step's projections into the other parity (late
        # emission = low priority: fills PE gaps without delaying rec MMs).
        if t + 1 <= T - 1:
            emit_proj(pid, t + 1, 1 - p)

    for rep in range(REPEAT):
        # All x chunks issued up front: the SP queue is in-order, so keeping
        # them ahead of the out DMAs (whose data waits are long) means an x
        # load can never be blocked behind an out store.
        for ci in range(XAHEAD, len(chunks)):
            emit_x_chunk(ci)
        emit_proj(1, 0, 0)  # layer-1 step 0 projections
        for s in range(T + LAG):
            p = s % 2
            t2 = s - LAG

            if s == LAG - 1:
                emit_proj(0, 0, (s + 1) % 2)  # layer-2 step 0 projections

            live0 = 0 <= t2 < T
            live1 = s < T
            # Phase A of both pairs first (rec matmuls + Tanh + cell pre-ops),
            # then phase B (tanh(c), h, c, next projections): pair1's Tanh
            # instructions cover pair0's cell-chain latency on the in-order
            # ACT stream, and vice versa.
            if live0:
                emit_pair_step(0, t2, p)
            if live1:
                emit_pair_step(1, s, p)
            if live0:
                emit_pair_tail(0, t2, p)
                if t2 % CH == CH - 1:
                    t0 = t2 - CH + 1
                    nc.sync.dma_start(
                        out[:, t0 : t0 + CH, :, :], hbuf[:, t0 : t0 + CH, :, :]
                    )
            if live1:
                emit_pair_tail(1, s, p)
def _build():
    nc = bacc.Bacc("TRN2", target_bir_lowering=False, debug=False, num_devices=NCORES)
    x_in = nc.dram_tensor("xT", [D, T, BC], BF16, kind="ExternalInput").ap()
    wk_in = nc.dram_tensor("wk", [4, D, 4 * U], BF16, kind="ExternalInput").ap()
    wrk_in = nc.dram_tensor("wrk", [4, U, 4 * U], BF16, kind="ExternalInput").ap()
    out = nc.dram_tensor("out", [U, T, 2, BC], BF16, kind="ExternalOutput").ap()
    out1 = None
    if DEBUG_L1:
        out1 = nc.dram_tensor("out1", [U, T, 2, BC], BF16, kind="ExternalOutput").ap()
    from contextlib import ExitStack

    with tile.TileContext(nc) as tc, ExitStack() as ctx:
        _emit(nc, tc, ctx, x_in, wk_in, wrk_in, out, out1)
    nc.compile()
    return nc


def _get_nc():
    if "nc" not in _CACHE:
        _CACHE["nc"] = _build()
    return _CACHE["nc"]


class _Runner:
    """Cached jitted executor (mirrors bass2jax.run_bass_via_pjrt, but the
    traced/jitted callable is built once and can be re-invoked with
    device-resident inputs for timing)."""

    def __init__(self, nc):
        import jax
        from jax.sharding import Mesh, PartitionSpec
        from jax.experimental.shard_map import shard_map
        from concourse.bass2jax import (
            _bass_exec_p,
            install_neuronx_cc_hook,
            partition_id_tensor,
        )
        import concourse.mybir as _mybir

        install_neuronx_cc_hook()
        self.jax = jax
        partition_name = (
            nc.partition_id_tensor.name if nc.partition_id_tensor else None
        )
        in_names, out_names, out_avals = [], [], []
        zero_outs = []
        for alloc in nc.m.functions[0].allocations:
            if not isinstance(alloc, _mybir.MemoryLocationSet):
                continue
            name = alloc.memorylocations[0].name
            if alloc.kind == "ExternalInput":
                if name != partition_name:
                    in_names.append(name)
            elif alloc.kind == "ExternalOutput":
                out_names.append(name)
                shape = tuple(alloc.tensor_shape)
                dtype = _mybir.dt.np(alloc.dtype)
                out_avals.append(jax.core.ShapedArray(shape, dtype))
                zero_outs.append(np.zeros(shape, dtype))
        self.in_names = list(in_names)
        self.out_names = out_names
        n_params = len(in_names)
        all_names = in_names + out_names
        if partition_name is not None:
            all_names = all_names + [partition_name]

        def _body(*args):
            operands = list(args)
            if partition_name is not None:
                operands.append(partition_id_tensor())
            outs = _bass_exec_p.bind(
                *operands,
                out_avals=tuple(out_avals),
                in_names=tuple(all_names),
                out_names=tuple(out_names),
                lowering_input_output_aliases=(),
                sim_require_finite=True,
                sim_require_nnan=True,
                nc=nc,
            )
            return tuple(outs)

        devices = jax.devices()[:NCORES]
        self.mesh = Mesh(np.asarray(devices), ("core",))
        in_specs = (PartitionSpec("core"),) * (n_params + len(out_names))
        out_specs = (PartitionSpec("core"),) * len(out_names)
        self.fn = jax.jit(
            shard_map(
                _body,
                mesh=self.mesh,
                in_specs=in_specs,
                out_specs=out_specs,
                check_rep=False,
            ),
            keep_unused=True,
        )
        self.zero_outs = zero_outs

    def put(self, in_maps):
        """Concatenate per-core inputs and move everything to device."""
        import jax
        from jax.sharding import NamedSharding, PartitionSpec

        sh = NamedSharding(self.mesh, PartitionSpec("core"))
        args = []
        for name in self.in_names:
            arr = np.concatenate([np.asarray(m[name]) for m in in_maps], axis=0)
            args.append(jax.device_put(arr, sh))
        for z in self.zero_outs:
            arr = np.concatenate([z] * NCORES, axis=0)
            args.append(jax.device_put(arr, sh))
        return args

    def run(self, args):
        outs = self.fn(*args)
        for o in outs:
            o.block_until_ready()
        return outs

    def gather(self, outs):
        res = []
        for c in range(NCORES):
            m = {}
            for i, name in enumerate(self.out_names):
                full = np.asarray(outs[i])
                n0 = full.shape[0] // NCORES
                m[name] = full[c * n0 : (c + 1) * n0]
            res.append(m)
        return res


def _get_runner():
    if "runner" not in _CACHE:
        _CACHE["runner"] = _Runner(_get_nc())
    return _CACHE["runner"]


def _pack_weights(fw_k, fw_rk, bw_k, bw_rk):
    """[unit, D, 4U] bf16 with gate columns permuted to [i, f, o, g].

    In TANH_FORM the tanh-argument halving for the sigmoid gates (i,f,o)
    and the h~=2h compensation (x0.5 on every weight fed by a hidden
    state) are folded in here.  All factors are powers of two, so the
    bf16 quantization is unchanged.
    """
    import ml_dtypes

    def perm(w):
        wg = w.reshape(w.shape[0], 4, U)
        return wg[:, GATE_SRC, :].reshape(w.shape[0], 4 * U)

    # units: 0=l2.fw 1=l2.bw 2=l1.fw 3=l1.bw
    wk = np.stack([perm(fw_k[1]), perm(bw_k[1]), perm(fw_k[0]), perm(bw_k[0])])
    wrk = np.stack([perm(fw_rk[1]), perm(bw_rk[1]), perm(fw_rk[0]), perm(bw_rk[0])])
    if TANH_FORM:
        col = np.concatenate(
            [np.full(3 * U, 0.5, np.float32), np.ones(U, np.float32)]
        )  # i,f,o halved; g unscaled
        hin = np.array([0.5, 0.5, 1.0, 1.0], np.float32)  # l2 proj input is h~=2h
        wk = wk * col[None, None, :] * hin[:, None, None]
        wrk = wrk * col[None, None, :] * 0.5  # every rec input is h~=2h
    return wk.astype(ml_dtypes.bfloat16), wrk.astype(ml_dtypes.bfloat16)


def make_in_maps(x, fw_k, fw_rk, bw_k, bw_rk):
    import ml_dtypes

    wk, wrk = _pack_weights(
        np.asarray(fw_k), np.asarray(fw_rk), np.asarray(bw_k), np.asarray(bw_rk)
    )
    x = np.asarray(x)
    in_maps = []
    for c in range(NCORES):
        xc = x[c * BC : (c + 1) * BC]  # [Bc, T, D]
        xT = np.ascontiguousarray(xc.transpose(2, 1, 0)).astype(ml_dtypes.bfloat16)
        in_maps.append({"xT": xT, "wk": wk, "wrk": wrk})
    return in_maps


def postprocess(res):
    # device h is h~=2h in TANH_FORM, so the merge scale absorbs the /2
    scale = 0.25 if TANH_FORM else 0.5
    outs = []
    for c in range(NCORES):
        o = np.asarray(res[c]["out"]).astype(np.float32)  # [U, T, 2, Bc]
        fw = o[:, :, 0, :].transpose(2, 1, 0)  # [Bc, T, U]
        bw = o[:, ::-1, 1, :].transpose(2, 1, 0)  # reverse raw bw order -> fwd time
        outs.append((fw + bw) * scale)
    return np.concatenate(outs, axis=0)


def kernel(x, fw_k, fw_rk, fw_b, bw_k, bw_rk, bw_b, **_unused):
    runner = _get_runner()
    in_maps = make_in_maps(x, fw_k, fw_rk, bw_k, bw_rk)
    args = runner.put(in_maps)
    outs = runner.run(args)
    return postprocess(runner.gather(outs))



# revision 27
# speedup vs baseline: 1.9223x; 1.9223x over previous
"""Bass/Trainium2 kernel for a 2-layer bidirectional LSTM (Keras semantics).

Problem: B=1024, T=200, D=U=128, 2 layers, merge_mode='ave', biases all 1.0.

Sharding: data-parallel over batch across 8 cores (Bc=128 per core).
Each core runs all 4 LSTM passes (fw/bw x 2 layers) on its batch slice as
two concurrent layer-pair wavefronts: the layer-2 pair lags the layer-1
pair by LAG steps, so both recurrences advance in parallel and fill each
other's pipeline gaps.

Layout: feature-major ("transposed") everywhere on device.  Hidden state h
is kept as [U, batch] tiles so it feeds the next step's matmul as the
stationary operand without any per-step transposes.  Each layer-pair's gate
pre-activations live in their own PSUM banks, double-buffered by step
parity, and the input projections are issued one step ahead (they do not
depend on the recurrent state) so only the 8 recurrent matmuls sit on each
step's serial chain.

The host pre-transposes x to [D, T, Bc] and pre-casts x/weights to bf16;
matmuls run bf16 with fp32 PSUM accumulation; cell state c stays fp32.
"""

import numpy as np

import concourse.bacc as bacc
import concourse.mybir as mybir
import concourse.tile as tile

B, T, D, U = 1024, 200, 128, 128
NCORES = 8
BC = B // NCORES
LAG = 4  # layer-2 wavefront lag in steps (must be >= 2)

F32 = mybir.dt.float32
BF16 = mybir.dt.bfloat16
SIGMOID = mybir.ActivationFunctionType.Sigmoid
TANH = mybir.ActivationFunctionType.Tanh
MULT = mybir.AluOpType.mult

# Units in the shared PSUM tile: 0=l2.fw 1=l2.bw 2=l1.fw 3=l1.bw.
# Gate bank order per unit is [i, f, o, g]; the host pre-permutes the
# source weights (gate order i,f,g,o) into this bank order.
GATE_SRC = [0, 1, 3, 2]

_CACHE = {}
DEBUG_L1 = False
REPEAT = 1  # emit the whole computation N times (device-time measurement)
SIG_SPLIT = True  # sigmoid [i,f] separate from [o]: shorter critical chain
# TANH_FORM: compute every gate with a single Tanh activation using
# sigmoid(y) = (tanh(y/2)+1)/2.  The per-gate argument scaling (y/2 for
# i,f,o) is folded into the weights on the host; the per-gate bias
# (0.5/0.5/0.5/1.0) is pre-added into PSUM by a K=1 matmul against a
# constant pattern; the (t+1)/2 fix-ups ride inside scalar_tensor_tensor
# ops, and h is stored doubled (h~ = 2h) with the compensation folded into
# the consuming weights and the host's final merge scale (0.25).
TANH_FORM = True
T_PER_UNIT = True  # one Tanh per unit (shorter chain) vs per pair (fewer insts)
L1_FIRST = False  # emit layer-1 pair before layer-2 within an iteration


def _emit(nc, tc, ctx, x_in, wk_in, wrk_in, out, out1=None):
    consts = ctx.enter_context(tc.tile_pool(name="consts", bufs=1))
    bigs = ctx.enter_context(tc.tile_pool(name="bigs", bufs=1))
    work = ctx.enter_context(tc.tile_pool(name="work", bufs=2))
    psum = ctx.enter_context(tc.tile_pool(name="psum", bufs=1, space="PSUM"))

    # Weights: [unit, D, 4U] with gates pre-permuted to [i,f,o,g] by the host.
    wk = consts.tile([128, 4, 4 * U], BF16, tag="wk")
    wrk = consts.tile([128, 4, 4 * U], BF16, tag="wrk")
    nc.sync.dma_start(wk[:], wk_in.rearrange("u p c -> p u c"))
    nc.sync.dma_start(wrk[:], wrk_in.rearrange("u p c -> p u c"))

    if TANH_FORM:
        # K=1 matmul operands for the per-gate-column bias pattern.
        ones1 = consts.tile([1, 128], BF16, tag="ones1")
        biasp = consts.tile([1, 4 * U], BF16, tag="biasp")
        nc.gpsimd.memset(ones1[:], 1.0)
        nc.gpsimd.memset(biasp[:, 0 : 3 * U], 0.5)
        nc.gpsimd.memset(biasp[:, 3 * U : 4 * U], 1.0)

    # Big persistent buffers.
    xT = bigs.tile([128, T, BC], BF16, tag="xT")          # x, feature-major
    hbuf = bigs.tile([128, T, 2, BC], BF16, tag="hbuf")   # l1 h, overwritten in place by l2 h
    gate_ps = psum.tile([128, 2, 4, 4 * U], F32, tag="ps")  # [parity, unit, i|f|o|g]

    # Input DMA: front/back interleaved 8-step chunks, emitted ahead of use.
    CH = 8
    chunks = []
    fr, bk = 0, T - CH
    while fr < bk:
        chunks.append(fr)
        chunks.append(bk)
        fr += CH
        bk -= CH
    if fr == bk:
        chunks.append(fr)

    def emit_x_chunk(ci):
        if ci < len(chunks):
            t0 = chunks[ci]
            nc.sync.dma_start(xT[:, t0 : t0 + CH, :], x_in[:, t0 : t0 + CH, :])

    XAHEAD = 4
    for ci in range(XAHEAD):
        emit_x_chunk(ci)

    # pair id 0 = layer 2 (units 0,1), pair id 1 = layer 1 (units 2,3)
    def proj_rhs(pid, u, t):
        if pid == 1:
            return xT[:, t, :] if u == 2 else xT[:, T - 1 - t, :]
        return hbuf[:, t, u, :]

    def emit_proj(pid, t, par):
        """Input projections for pair `pid` step `t` into parity `par`.

        The whole bank (4 gate projections + 4 recurrent matmuls of the same
        step) forms ONE psum accumulation group: start on the first proj,
        stop on the last rec (or last proj for step 0, which has no recs).
        """
        units = (2, 3) if pid == 1 else (0, 1)
        for u in units:
            rhs = proj_rhs(pid, u, t)
            for g in range(4):
                dst = gate_ps[:, par, u, g * U : (g + 1) * U]
                w = slice(g * U, (g + 1) * U)
                stop = t == 0 and g == 3 and not TANH_FORM
                nc.tensor.matmul(
                    dst, wk[:, u, w], rhs, start=(g == 0), stop=stop
                )
            if TANH_FORM:
                nc.tensor.matmul(
                    gate_ps[:, par, u, :],
                    ones1[:],
                    biasp[:],
                    start=False,
                    stop=(t == 0),
                )

    def emit_pair_step(pid, t, p):
        units = (2, 3) if pid == 1 else (0, 1)
        ulo = units[0]
        tag = f"p{pid}"

        # --- recurrent matmuls: the only PE work on the step's serial chain.
        if t > 0:
            for u in units:
                rhs = hbuf[:, t - 1, u - ulo, :]
                for g in range(4):
                    dst = gate_ps[:, p, u, g * U : (g + 1) * U]
                    w = slice(g * U, (g + 1) * U)
                    nc.tensor.matmul(
                        dst, wrk[:, u, w], rhs, start=False, stop=(g == 3)
                    )

        if TANH_FORM:
            # One Tanh covers all four gates: i',f',o' are tanh-form
            # sigmoids ((t+1)/2 folded into STT ops below), g is final.
            th = work.tile([128, 2, 4 * U], F32, tag="th" + tag, bufs=3)
            ps_pair = gate_ps[:, p, ulo : ulo + 2, :]
            ADD = mybir.AluOpType.add
            # s = 2*c = (f'+1)*c_prev + (i'+1)*g, computed per unit so each
            # unit's cell math starts right after its own Tanh.
            s_t = work.tile([128, 2, U], F32, tag="s" + tag, bufs=3)
            c_prev = _CACHE["c_prev" + tag] if t > 0 else None
            if t > 0:
                e1 = work.tile([128, 2, U], F32, tag="e1" + tag)
                e2 = work.tile([128, 2, U], F32, tag="e2" + tag)
            if not T_PER_UNIT:
                nc.scalar.activation(th[:], ps_pair[:], TANH)
            for ui in range(2):
                if T_PER_UNIT:
                    nc.scalar.activation(th[:, ui, :], ps_pair[:, ui, :], TANH)
                ip = th[:, ui, 0:U]
                fp_ = th[:, ui, U : 2 * U]
                gp = th[:, ui, 3 * U : 4 * U]
                su = s_t[:, ui, :]
                if t == 0:
                    nc.vector.scalar_tensor_tensor(su, ip, 1.0, gp, ADD, MULT)
                else:
                    # STT is DVE-only (walrus rejects it on Pool); the plain
                    # add runs on GPSIMD to keep DVE off the chain tail.
                    nc.vector.scalar_tensor_tensor(
                        e2[:, ui, :], ip, 1.0, gp, ADD, MULT
                    )
                    nc.vector.scalar_tensor_tensor(
                        e1[:, ui, :], fp_, 1.0, c_prev[:, ui, :], ADD, MULT
                    )
                    nc.gpsimd.tensor_add(su, e1[:, ui, :], e2[:, ui, :])
            _CACHE["th" + tag] = th
            _CACHE["s" + tag] = s_t
            return

    def emit_pair_tail(pid, t, p):
        """Phase B of a step: tanh(c), h, c.  Emitted after BOTH pairs'
        phase A so the other pair's Tanh instructions cover this pair's
        DVE/GPSIMD cell-chain latency on the in-order ACT stream."""
        tag = f"p{pid}"
        th = _CACHE["th" + tag]
        s_t = _CACHE["s" + tag]
        ADD = mybir.AluOpType.add
        c_new = work.tile([128, 2, U], F32, tag="c" + tag)
        tanc = work.tile([128, 2, U], F32, tag="tanc" + tag, bufs=3)
        nc.scalar.activation(tanc[:], s_t[:], TANH, scale=0.5)
        # h~ = 2h = (o'+1)*tanh(c) -> bf16 layer output / next input,
        # written per unit so each unit's next rec matmuls start as soon
        # as its own half is stored.
        for ui in range(2):
            nc.vector.scalar_tensor_tensor(
                hbuf[:, t, ui, :],
                th[:, ui, 2 * U : 3 * U],
                1.0,
                tanc[:, ui, :],
                ADD,
                MULT,
            )
        # true c for the next step (off the h critical path)
        nc.vector.tensor_scalar_mul(c_new[:], s_t[:], 0.5)
        _CACHE["c_prev" + tag] = c_new
        # --- prefetch next step's projections into the other parity (late
        # emission = low priority: fills PE gaps without delaying rec MMs).
        if t + 1 <= T - 1:
            emit_proj(pid, t + 1, 1 - p)

    for rep in range(REPEAT):
        # All x chunks issued up front: the SP queue is in-order, so keeping
        # them ahead of the out DMAs (whose data waits are long) means an x
        # load can never be blocked behind an out store.
        for ci in range(XAHEAD, len(chunks)):
            emit_x_chunk(ci)
        emit_proj(1, 0, 0)  # layer-1 step 0 projections
        for s in range(T + LAG):
            p = s % 2
            t2 = s - LAG

            if s == LAG - 1:
                emit_proj(0, 0, (s + 1) % 2)  # layer-2 step 0 projections

            if 0 <= t2 < T:
                emit_pair_step(0, t2, p)
                emit_pair_tail(0, t2, p)
                if t2 % CH == CH - 1:
                    t0 = t2 - CH + 1
                    nc.sync.dma_start(
                        out[:, t0 : t0 + CH, :, :], hbuf[:, t0 : t0 + CH, :, :]
                    )
            if s < T:
                emit_pair_step(1, s, p)
                emit_pair_tail(1, s, p)
def _build():
    nc = bacc.Bacc("TRN2", target_bir_lowering=False, debug=False, num_devices=NCORES)
    x_in = nc.dram_tensor("xT", [D, T, BC], BF16, kind="ExternalInput").ap()
    wk_in = nc.dram_tensor("wk", [4, D, 4 * U], BF16, kind="ExternalInput").ap()
    wrk_in = nc.dram_tensor("wrk", [4, U, 4 * U], BF16, kind="ExternalInput").ap()
    out = nc.dram_tensor("out", [U, T, 2, BC], BF16, kind="ExternalOutput").ap()
    out1 = None
    if DEBUG_L1:
        out1 = nc.dram_tensor("out1", [U, T, 2, BC], BF16, kind="ExternalOutput").ap()
    from contextlib import ExitStack

    with tile.TileContext(nc) as tc, ExitStack() as ctx:
        _emit(nc, tc, ctx, x_in, wk_in, wrk_in, out, out1)
    nc.compile()
    return nc


def _get_nc():
    if "nc" not in _CACHE:
        _CACHE["nc"] = _build()
    return _CACHE["nc"]


class _Runner:
    """Cached jitted executor (mirrors bass2jax.run_bass_via_pjrt, but the
    traced/jitted callable is built once and can be re-invoked with
    device-resident inputs for timing)."""

    def __init__(self, nc):
        import jax
        from jax.sharding import Mesh, PartitionSpec
        from jax.experimental.shard_map import shard_map
        from concourse.bass2jax import (
            _bass_exec_p,
            install_neuronx_cc_hook,
            partition_id_tensor,
        )
        import concourse.mybir as _mybir

        install_neuronx_cc_hook()
        self.jax = jax
        partition_name = (
            nc.partition_id_tensor.name if nc.partition_id_tensor else None
        )
        in_names, out_names, out_avals = [], [], []
        zero_outs = []
        for alloc in nc.m.functions[0].allocations:
            if not isinstance(alloc, _mybir.MemoryLocationSet):
                continue
            name = alloc.memorylocations[0].name
            if alloc.kind == "ExternalInput":
                if name != partition_name:
                    in_names.append(name)
            elif alloc.kind == "ExternalOutput":
                out_names.append(name)
                shape = tuple(alloc.tensor_shape)
                dtype = _mybir.dt.np(alloc.dtype)
                out_avals.append(jax.core.ShapedArray(shape, dtype))
                zero_outs.append(np.zeros(shape, dtype))
        self.in_names = list(in_names)
        self.out_names = out_names
        n_params = len(in_names)
        all_names = in_names + out_names
        if partition_name is not None:
            all_names = all_names + [partition_name]

        def _body(*args):
            operands = list(args)
            if partition_name is not None:
                operands.append(partition_id_tensor())
            outs = _bass_exec_p.bind(
                *operands,
                out_avals=tuple(out_avals),
                in_names=tuple(all_names),
                out_names=tuple(out_names),
                lowering_input_output_aliases=(),
                sim_require_finite=True,
                sim_require_nnan=True,
                nc=nc,
            )
            return tuple(outs)

        devices = jax.devices()[:NCORES]
        self.mesh = Mesh(np.asarray(devices), ("core",))
        in_specs = (PartitionSpec("core"),) * (n_params + len(out_names))
        out_specs = (PartitionSpec("core"),) * len(out_names)
        self.fn = jax.jit(
            shard_map(
                _body,
                mesh=self.mesh,
                in_specs=in_specs,
                out_specs=out_specs,
                check_rep=False,
            ),
            keep_unused=True,
        )
        self.zero_outs = zero_outs

    def put(self, in_maps):
        """Concatenate per-core inputs and move everything to device."""
        import jax
        from jax.sharding import NamedSharding, PartitionSpec

        sh = NamedSharding(self.mesh, PartitionSpec("core"))
        args = []
        for name in self.in_names:
            arr = np.concatenate([np.asarray(m[name]) for m in in_maps], axis=0)
            args.append(jax.device_put(arr, sh))
        for z in self.zero_outs:
            arr = np.concatenate([z] * NCORES, axis=0)
            args.append(jax.device_put(arr, sh))
        return args

    def run(self, args):
        outs = self.fn(*args)
        for o in outs:
            o.block_until_ready()
        return outs

    def gather(self, outs):
        res = []
        for c in range(NCORES):
            m = {}
            for i, name in enumerate(self.out_names):
                full = np.asarray(outs[i])
                n0 = full.shape[0] // NCORES
                m[name] = full[c * n0 : (c + 1) * n0]
            res.append(m)
        return res


def _get_runner():
    if "runner" not in _CACHE:
        _CACHE["runner"] = _Runner(_get_nc())
    return _CACHE["runner"]


def _pack_weights(fw_k, fw_rk, bw_k, bw_rk):
    """[unit, D, 4U] bf16 with gate columns permuted to [i, f, o, g].

    In TANH_FORM the tanh-argument halving for the sigmoid gates (i,f,o)
    and the h~=2h compensation (x0.5 on every weight fed by a hidden
    state) are folded in here.  All factors are powers of two, so the
    bf16 quantization is unchanged.
    """
    import ml_dtypes

    def perm(w):
        wg = w.reshape(w.shape[0], 4, U)
        return wg[:, GATE_SRC, :].reshape(w.shape[0], 4 * U)

    # units: 0=l2.fw 1=l2.bw 2=l1.fw 3=l1.bw
    wk = np.stack([perm(fw_k[1]), perm(bw_k[1]), perm(fw_k[0]), perm(bw_k[0])])
    wrk = np.stack([perm(fw_rk[1]), perm(bw_rk[1]), perm(fw_rk[0]), perm(bw_rk[0])])
    if TANH_FORM:
        col = np.concatenate(
            [np.full(3 * U, 0.5, np.float32), np.ones(U, np.float32)]
        )  # i,f,o halved; g unscaled
        hin = np.array([0.5, 0.5, 1.0, 1.0], np.float32)  # l2 proj input is h~=2h
        wk = wk * col[None, None, :] * hin[:, None, None]
        wrk = wrk * col[None, None, :] * 0.5  # every rec input is h~=2h
    return wk.astype(ml_dtypes.bfloat16), wrk.astype(ml_dtypes.bfloat16)


def make_in_maps(x, fw_k, fw_rk, bw_k, bw_rk):
    import ml_dtypes

    wk, wrk = _pack_weights(
        np.asarray(fw_k), np.asarray(fw_rk), np.asarray(bw_k), np.asarray(bw_rk)
    )
    x = np.asarray(x)
    in_maps = []
    for c in range(NCORES):
        xc = x[c * BC : (c + 1) * BC]  # [Bc, T, D]
        xT = np.ascontiguousarray(xc.transpose(2, 1, 0)).astype(ml_dtypes.bfloat16)
        in_maps.append({"xT": xT, "wk": wk, "wrk": wrk})
    return in_maps


def postprocess(res):
    # device h is h~=2h in TANH_FORM, so the merge scale absorbs the /2
    scale = 0.25 if TANH_FORM else 0.5
    outs = []
    for c in range(NCORES):
        o = np.asarray(res[c]["out"]).astype(np.float32)  # [U, T, 2, Bc]
        fw = o[:, :, 0, :].transpose(2, 1, 0)  # [Bc, T, U]
        bw = o[:, ::-1, 1, :].transpose(2, 1, 0)  # reverse raw bw order -> fwd time
        outs.append((fw + bw) * scale)
    return np.concatenate(outs, axis=0)


def kernel(x, fw_k, fw_rk, fw_b, bw_k, bw_rk, bw_b, **_unused):
    runner = _get_runner()
    in_maps = make_in_maps(x, fw_k, fw_rk, bw_k, bw_rk)
    args = runner.put(in_maps)
    outs = runner.run(args)
    return postprocess(runner.gather(outs))



# revision 29
# speedup vs baseline: 7.6655x; 3.9877x over previous
"""Bass/Trainium2 kernel for a 2-layer bidirectional LSTM (Keras semantics).

Problem: B=1024, T=200, D=U=128, 2 layers, merge_mode='ave', biases all 1.0.

Sharding: data-parallel over batch across 8 cores (Bc=128 per core).
Each core runs all 4 LSTM passes (fw/bw x 2 layers) on its batch slice as
two concurrent layer-pair wavefronts: the layer-2 pair lags the layer-1
pair by LAG steps, so both recurrences advance in parallel and fill each
other's pipeline gaps.

Layout: feature-major ("transposed") everywhere on device.  Hidden state h
is kept as [U, batch] tiles so it feeds the next step's matmul as the
stationary operand without any per-step transposes.  Each layer-pair's gate
pre-activations live in their own PSUM banks, double-buffered by step
parity, and the input projections are issued one step ahead (they do not
depend on the recurrent state) so only the 8 recurrent matmuls sit on each
step's serial chain.

The host pre-transposes x to [D, T, Bc] and pre-casts x/weights to bf16;
matmuls run bf16 with fp32 PSUM accumulation; cell state c stays fp32.
"""

import numpy as np

import concourse.bacc as bacc
import concourse.mybir as mybir
import concourse.tile as tile

B, T, D, U = 1024, 200, 128, 128
NCORES = 8
BC = B // NCORES
LAG = 4  # layer-2 wavefront lag in steps (must be >= 2)

F32 = mybir.dt.float32
BF16 = mybir.dt.bfloat16
SIGMOID = mybir.ActivationFunctionType.Sigmoid
TANH = mybir.ActivationFunctionType.Tanh
MULT = mybir.AluOpType.mult

# Units in the shared PSUM tile: 0=l2.fw 1=l2.bw 2=l1.fw 3=l1.bw.
# Gate bank order per unit is [i, f, o, g]; the host pre-permutes the
# source weights (gate order i,f,g,o) into this bank order.
GATE_SRC = [0, 1, 3, 2]

_CACHE = {}
DEBUG_L1 = False
REPEAT = 1  # emit the whole computation N times (device-time measurement)
SIG_SPLIT = True  # sigmoid [i,f] separate from [o]: shorter critical chain
# TANH_FORM: compute every gate with a single Tanh activation using
# sigmoid(y) = (tanh(y/2)+1)/2.  The per-gate argument scaling (y/2 for
# i,f,o) is folded into the weights on the host; the per-gate bias
# (0.5/0.5/0.5/1.0) is pre-added into PSUM by a K=1 matmul against a
# constant pattern; the (t+1)/2 fix-ups ride inside scalar_tensor_tensor
# ops, and h is stored doubled (h~ = 2h) with the compensation folded into
# the consuming weights and the host's final merge scale (0.25).
TANH_FORM = True
T_PER_UNIT = True  # one Tanh per unit (shorter chain) vs per pair (fewer insts)
L1_FIRST = False  # emit layer-1 pair before layer-2 within an iteration


def _emit(nc, tc, ctx, x_in, wk_in, wrk_in, out, out1=None):
    consts = ctx.enter_context(tc.tile_pool(name="consts", bufs=1))
    bigs = ctx.enter_context(tc.tile_pool(name="bigs", bufs=1))
    work = ctx.enter_context(tc.tile_pool(name="work", bufs=2))
    psum = ctx.enter_context(tc.tile_pool(name="psum", bufs=1, space="PSUM"))

    # Weights: [unit, D, 4U] with gates pre-permuted to [i,f,o,g] by the host.
    wk = consts.tile([128, 4, 4 * U], BF16, tag="wk")
    wrk = consts.tile([128, 4, 4 * U], BF16, tag="wrk")
    nc.sync.dma_start(wk[:], wk_in.rearrange("u p c -> p u c"))
    nc.sync.dma_start(wrk[:], wrk_in.rearrange("u p c -> p u c"))

    if TANH_FORM:
        # K=1 matmul operands for the per-gate-column bias pattern.
        ones1 = consts.tile([1, 128], BF16, tag="ones1")
        biasp = consts.tile([1, 4 * U], BF16, tag="biasp")
        nc.gpsimd.memset(ones1[:], 1.0)
        nc.gpsimd.memset(biasp[:, 0 : 3 * U], 0.5)
        nc.gpsimd.memset(biasp[:, 3 * U : 4 * U], 1.0)

    # Big persistent buffers.
    xT = bigs.tile([128, T, BC], BF16, tag="xT")          # x, feature-major
    hbuf = bigs.tile([128, T, 2, BC], BF16, tag="hbuf")   # l1 h, overwritten in place by l2 h
    gate_ps = psum.tile([128, 2, 4, 4 * U], F32, tag="ps")  # [parity, unit, i|f|o|g]

    # Input DMA: front/back interleaved 8-step chunks, emitted ahead of use.
    CH = 8
    chunks = []
    fr, bk = 0, T - CH
    while fr < bk:
        chunks.append(fr)
        chunks.append(bk)
        fr += CH
        bk -= CH
    if fr == bk:
        chunks.append(fr)

    def emit_x_chunk(ci):
        if ci < len(chunks):
            t0 = chunks[ci]
            nc.sync.dma_start(xT[:, t0 : t0 + CH, :], x_in[:, t0 : t0 + CH, :])

    XAHEAD = 4
    for ci in range(XAHEAD):
        emit_x_chunk(ci)

    # pair id 0 = layer 2 (units 0,1), pair id 1 = layer 1 (units 2,3)
    def proj_rhs(pid, u, t):
        if pid == 1:
            return xT[:, t, :] if u == 2 else xT[:, T - 1 - t, :]
        return hbuf[:, t, u, :]

    def emit_proj(pid, t, par):
        """Input projections for pair `pid` step `t` into parity `par`.

        The whole bank (4 gate projections + 4 recurrent matmuls of the same
        step) forms ONE psum accumulation group: start on the first proj,
        stop on the last rec (or last proj for step 0, which has no recs).
        """
        units = (2, 3) if pid == 1 else (0, 1)
        for u in units:
            rhs = proj_rhs(pid, u, t)
            for g in range(4):
                dst = gate_ps[:, par, u, g * U : (g + 1) * U]
                w = slice(g * U, (g + 1) * U)
                stop = t == 0 and g == 3 and not TANH_FORM
                nc.tensor.matmul(
                    dst, wk[:, u, w], rhs, start=(g == 0), stop=stop
                )
            if TANH_FORM:
                nc.tensor.matmul(
                    gate_ps[:, par, u, :],
                    ones1[:],
                    biasp[:],
                    start=False,
                    stop=(t == 0),
                )

    def emit_pair_step(pid, t, p):
        units = (2, 3) if pid == 1 else (0, 1)
        ulo = units[0]
        tag = f"p{pid}"

        # --- recurrent matmuls: the only PE work on the step's serial chain.
        if t > 0:
            for u in units:
                rhs = hbuf[:, t - 1, u - ulo, :]
                for g in range(4):
                    dst = gate_ps[:, p, u, g * U : (g + 1) * U]
                    w = slice(g * U, (g + 1) * U)
                    nc.tensor.matmul(
                        dst, wrk[:, u, w], rhs, start=False, stop=(g == 3)
                    )

        if TANH_FORM:
            # One Tanh covers all four gates: i',f',o' are tanh-form
            # sigmoids ((t+1)/2 folded into STT ops below), g is final.
            th = work.tile([128, 2, 4 * U], F32, tag="th" + tag, bufs=3)
            ps_pair = gate_ps[:, p, ulo : ulo + 2, :]
            ADD = mybir.AluOpType.add
            # s = 2*c = (f'+1)*c_prev + (i'+1)*g, computed per unit so each
            # unit's cell math starts right after its own Tanh.
            s_t = work.tile([128, 2, U], F32, tag="s" + tag, bufs=3)
            c_prev = _CACHE["c_prev" + tag] if t > 0 else None
            if t > 0:
                e1 = work.tile([128, 2, U], F32, tag="e1" + tag)
                e2 = work.tile([128, 2, U], F32, tag="e2" + tag)
            if not T_PER_UNIT:
                nc.scalar.activation(th[:], ps_pair[:], TANH)
            for ui in range(2):
                if T_PER_UNIT:
                    nc.scalar.activation(th[:, ui, :], ps_pair[:, ui, :], TANH)
                ip = th[:, ui, 0:U]
                fp_ = th[:, ui, U : 2 * U]
                gp = th[:, ui, 3 * U : 4 * U]
                su = s_t[:, ui, :]
                if t == 0:
                    nc.vector.scalar_tensor_tensor(su, ip, 1.0, gp, ADD, MULT)
                else:
                    # STT is DVE-only (walrus rejects it on Pool); the plain
                    # add runs on GPSIMD to keep DVE off the chain tail.
                    nc.vector.scalar_tensor_tensor(
                        e2[:, ui, :], ip, 1.0, gp, ADD, MULT
                    )
                    nc.vector.scalar_tensor_tensor(
                        e1[:, ui, :], fp_, 1.0, c_prev[:, ui, :], ADD, MULT
                    )
                    nc.gpsimd.tensor_add(su, e1[:, ui, :], e2[:, ui, :])
            _CACHE["th" + tag] = th
            _CACHE["s" + tag] = s_t
            return

    def emit_pair_tail(pid, t, p):
        """Phase B of a step: tanh(c), h, c.  Emitted after BOTH pairs'
        phase A so the other pair's Tanh instructions cover this pair's
        DVE/GPSIMD cell-chain latency on the in-order ACT stream."""
        tag = f"p{pid}"
        th = _CACHE["th" + tag]
        s_t = _CACHE["s" + tag]
        ADD = mybir.AluOpType.add
        c_new = work.tile([128, 2, U], F32, tag="c" + tag)
        tanc = work.tile([128, 2, U], F32, tag="tanc" + tag, bufs=3)
        nc.scalar.activation(tanc[:], s_t[:], TANH, scale=0.5)
        # h~ = 2h = (o'+1)*tanh(c) -> bf16 layer output / next input,
        # written per unit so each unit's next rec matmuls start as soon
        # as its own half is stored.
        for ui in range(2):
            nc.vector.scalar_tensor_tensor(
                hbuf[:, t, ui, :],
                th[:, ui, 2 * U : 3 * U],
                1.0,
                tanc[:, ui, :],
                ADD,
                MULT,
            )
        # true c for the next step (off the h critical path)
        nc.vector.tensor_scalar_mul(c_new[:], s_t[:], 0.5)
        _CACHE["c_prev" + tag] = c_new
        # --- prefetch next step's projections into the other parity (late
        # emission = low priority: fills PE gaps without delaying rec MMs).
        if t + 1 <= T - 1:
            emit_proj(pid, t + 1, 1 - p)

    for rep in range(REPEAT):
        # All x chunks issued up front: the SP queue is in-order, so keeping
        # them ahead of the out DMAs (whose data waits are long) means an x
        # load can never be blocked behind an out store.
        for ci in range(XAHEAD, len(chunks)):
            emit_x_chunk(ci)
        emit_proj(1, 0, 0)  # layer-1 step 0 projections
        for s in range(T + LAG):
            p = s % 2
            t2 = s - LAG

            if s == LAG - 1:
                emit_proj(0, 0, (s + 1) % 2)  # layer-2 step 0 projections

            if 0 <= t2 < T:
                emit_pair_step(0, t2, p)
                emit_pair_tail(0, t2, p)
                if t2 % CH == CH - 1:
                    t0 = t2 - CH + 1
                    nc.sync.dma_start(
                        out[:, t0 : t0 + CH, :, :], hbuf[:, t0 : t0 + CH, :, :]
                    )
            if s < T:
                emit_pair_step(1, s, p)
                emit_pair_tail(1, s, p)
def _build():
    nc = bacc.Bacc("TRN2", target_bir_lowering=False, debug=False, num_devices=NCORES)
    x_in = nc.dram_tensor("xT", [D, T, BC], BF16, kind="ExternalInput").ap()
    wk_in = nc.dram_tensor("wk", [4, D, 4 * U], BF16, kind="ExternalInput").ap()
    wrk_in = nc.dram_tensor("wrk", [4, U, 4 * U], BF16, kind="ExternalInput").ap()
    out = nc.dram_tensor("out", [U, T, 2, BC], BF16, kind="ExternalOutput").ap()
    out1 = None
    if DEBUG_L1:
        out1 = nc.dram_tensor("out1", [U, T, 2, BC], BF16, kind="ExternalOutput").ap()
    from contextlib import ExitStack

    with tile.TileContext(nc) as tc, ExitStack() as ctx:
        _emit(nc, tc, ctx, x_in, wk_in, wrk_in, out, out1)
    nc.compile()
    return nc


def _get_nc():
    if "nc" not in _CACHE:
        _CACHE["nc"] = _build()
    return _CACHE["nc"]


class _Runner:
    """Cached jitted executor (mirrors bass2jax.run_bass_via_pjrt, but the
    traced/jitted callable is built once and can be re-invoked with
    device-resident inputs for timing)."""

    def __init__(self, nc):
        import jax
        from jax.sharding import Mesh, PartitionSpec
        from jax.experimental.shard_map import shard_map
        from concourse.bass2jax import (
            _bass_exec_p,
            install_neuronx_cc_hook,
            partition_id_tensor,
        )
        import concourse.mybir as _mybir

        install_neuronx_cc_hook()
        self.jax = jax
        partition_name = (
            nc.partition_id_tensor.name if nc.partition_id_tensor else None
        )
        in_names, out_names, out_avals = [], [], []
        zero_outs = []
        for alloc in nc.m.functions[0].allocations:
            if not isinstance(alloc, _mybir.MemoryLocationSet):
                continue
            name = alloc.memorylocations[0].name
            if alloc.kind == "ExternalInput":
                if name != partition_name:
                    in_names.append(name)
            elif alloc.kind == "ExternalOutput":
                out_names.append(name)
                shape = tuple(alloc.tensor_shape)
                dtype = _mybir.dt.np(alloc.dtype)
                out_avals.append(jax.core.ShapedArray(shape, dtype))
                zero_outs.append(np.zeros(shape, dtype))
        self.in_names = list(in_names)
        self.out_names = out_names
        n_params = len(in_names)
        all_names = in_names + out_names
        if partition_name is not None:
            all_names = all_names + [partition_name]

        def _body(*args):
            operands = list(args)
            if partition_name is not None:
                operands.append(partition_id_tensor())
            outs = _bass_exec_p.bind(
                *operands,
                out_avals=tuple(out_avals),
                in_names=tuple(all_names),
                out_names=tuple(out_names),
                lowering_input_output_aliases=(),
                sim_require_finite=True,
                sim_require_nnan=True,
                nc=nc,
            )
            return tuple(outs)

        devices = jax.devices()[:NCORES]
        self.mesh = Mesh(np.asarray(devices), ("core",))
        in_specs = (PartitionSpec("core"),) * (n_params + len(out_names))
        out_specs = (PartitionSpec("core"),) * len(out_names)
        self.fn = jax.jit(
            shard_map(
                _body,
                mesh=self.mesh,
                in_specs=in_specs,
                out_specs=out_specs,
                check_rep=False,
            ),
            keep_unused=True,
        )
        self.zero_outs = zero_outs

    def put(self, in_maps):
        """Concatenate per-core inputs and move everything to device."""
        import jax
        from jax.sharding import NamedSharding, PartitionSpec

        sh = NamedSharding(self.mesh, PartitionSpec("core"))
        args = []
        for name in self.in_names:
            arr = np.concatenate([np.asarray(m[name]) for m in in_maps], axis=0)
            args.append(jax.device_put(arr, sh))
        for z in self.zero_outs:
            arr = np.concatenate([z] * NCORES, axis=0)
            args.append(jax.device_put(arr, sh))
        return args

    def run(self, args):
        outs = self.fn(*args)
        for o in outs:
            o.block_until_ready()
        return outs

    def gather(self, outs):
        res = []
        for c in range(NCORES):
            m = {}
            for i, name in enumerate(self.out_names):
                full = np.asarray(outs[i])
                n0 = full.shape[0] // NCORES
                m[name] = full[c * n0 : (c + 1) * n0]
            res.append(m)
        return res


def _get_runner():
    if "runner" not in _CACHE:
        _CACHE["runner"] = _Runner(_get_nc())
    return _CACHE["runner"]


def _pack_weights(fw_k, fw_rk, bw_k, bw_rk):
    """[unit, D, 4U] bf16 with gate columns permuted to [i, f, o, g].

    In TANH_FORM the tanh-argument halving for the sigmoid gates (i,f,o)
    and the h~=2h compensation (x0.5 on every weight fed by a hidden
    state) are folded in here.  All factors are powers of two, so the
    bf16 quantization is unchanged.
    """
    import ml_dtypes

    def perm(w):
        wg = w.reshape(w.shape[0], 4, U)
        return wg[:, GATE_SRC, :].reshape(w.shape[0], 4 * U)

    # units: 0=l2.fw 1=l2.bw 2=l1.fw 3=l1.bw
    wk = np.stack([perm(fw_k[1]), perm(bw_k[1]), perm(fw_k[0]), perm(bw_k[0])])
    wrk = np.stack([perm(fw_rk[1]), perm(bw_rk[1]), perm(fw_rk[0]), perm(bw_rk[0])])
    if TANH_FORM:
        col = np.concatenate(
            [np.full(3 * U, 0.5, np.float32), np.ones(U, np.float32)]
        )  # i,f,o halved; g unscaled
        hin = np.array([0.5, 0.5, 1.0, 1.0], np.float32)  # l2 proj input is h~=2h
        wk = wk * col[None, None, :] * hin[:, None, None]
        wrk = wrk * col[None, None, :] * 0.5  # every rec input is h~=2h
    return wk.astype(ml_dtypes.bfloat16), wrk.astype(ml_dtypes.bfloat16)


def make_in_maps(x, fw_k, fw_rk, bw_k, bw_rk):
    import ml_dtypes

    wk, wrk = _pack_weights(
        np.asarray(fw_k), np.asarray(fw_rk), np.asarray(bw_k), np.asarray(bw_rk)
    )
    x = np.asarray(x)
    in_maps = []
    for c in range(NCORES):
        xc = x[c * BC : (c + 1) * BC]  # [Bc, T, D]
        xT = np.ascontiguousarray(xc.transpose(2, 1, 0)).astype(ml_dtypes.bfloat16)
        in_maps.append({"xT": xT, "wk": wk, "wrk": wrk})
    return in_maps


def postprocess(res):
    # device h is h~=2h in TANH_FORM, so the merge scale absorbs the /2
    scale = 0.25 if TANH_FORM else 0.5
    outs = []
    for c in range(NCORES):
        o = np.asarray(res[c]["out"]).astype(np.float32)  # [U, T, 2, Bc]
        fw = o[:, :, 0, :].transpose(2, 1, 0)  # [Bc, T, U]
        bw = o[:, ::-1, 1, :].transpose(2, 1, 0)  # reverse raw bw order -> fwd time
        outs.append((fw + bw) * scale)
    return np.concatenate(outs, axis=0)


def kernel(x, fw_k, fw_rk, fw_b, bw_k, bw_rk, bw_b, **_unused):
    runner = _get_runner()
    in_maps = make_in_maps(x, fw_k, fw_rk, bw_k, bw_rk)
    args = runner.put(in_maps)
    outs = runner.run(args)
    return postprocess(runner.gather(outs))

